# revision 1
# baseline (speedup 1.0000x reference)
"""Trainium2 Bass kernel for nn_DenseProduct (num_factors=2).

Computes, for input x of shape (128, 16, 64, 32) f32:
    out[s, d, b, i*32+j] = x[2s, d, b, i] + x[2s+1, d, b, j]
with output shape (64, 16, 64, 1024) f32.

Sharding: scope axis (dim 0) across 8 NeuronCores — core c gets input
scopes [16c, 16c+16) and produces output scopes [8c, 8c+8), a contiguous
33.5 MB slice of the output per core.

Per-core layout: SBUF partition p = d*8 + b_hi (d in [0,16), b_hi in [0,8),
b = 8*b_hi + b_lo). This makes the input DMA read contiguous 1 KB runs and
the output DMA write one contiguous 4 MB region per scope (32 KB per
partition). The whole outer-sum for one scope is a single DVE tensor_tensor
with stride-0 (broadcast) free dims:
    out[p, (bl, i, j)] = A[p, (bl, i)] + B[p, (bl, j)]
"""

import numpy as np

_S_IN = 128        # total input scopes
_NF = 2            # num_factors (hardcoded)
_S_OUT = _S_IN // _NF
_D = 16
_B = 64
_N = 32
_N_CORES = 8
_SIN_LOC = _S_IN // _N_CORES   # 16 input scopes per core
_S_LOC = _S_OUT // _N_CORES    # 8 output scopes per core
_P = 128
_BH = 8
_BL = 8
_FREE_IN = _BL * _N            # 256
_FREE_OUT = _BL * _N * _N      # 8192

_CACHE = {}
LAST_RESULTS = None  # BassKernelResults of the most recent run (for profiling)


def _build_bass():
    import concourse.bacc as bacc
    import concourse.mybir as mybir
    from concourse.tile import TileContext

    nc = bacc.Bacc("TRN2", target_bir_lowering=False, debug=False,
                   num_devices=_N_CORES)
    x = nc.dram_tensor("x", [_SIN_LOC, _D, _B, _N], mybir.dt.float32,
                       kind="ExternalInput").ap()
    out = nc.dram_tensor("out", [_S_LOC, _D, _B, _N * _N], mybir.dt.float32,
                         kind="ExternalOutput").ap()

    with TileContext(nc) as tc:
        with tc.tile_pool(name="inp", bufs=_S_LOC) as in_pool, \
             tc.tile_pool(name="head", bufs=1) as head_pool, \
             tc.tile_pool(name="outp", bufs=4) as out_pool:
            # x[s_in, d, 8*bh+bl, n] -> partition (d, bh), free (s_in, bl, n)
            xr = x.rearrange("s d (bh bl) n -> (d bh) s (bl n)", bh=_BH)
            # tiny head tile: bl=0 strip of both factors of scope 0, so the
            # very first compute piece (and with it the output DMA stream)
            # starts ~1.5us before the full scope-0 input lands
            ht = head_pool.tile([_P, 2 * _N], mybir.dt.float32)
            nc.sync.dma_start(out=ht[:, :].rearrange("p (s f) -> p s f", s=2),
                              in_=xr[:, 0:2, 0:_N])
            in_tiles = []
            for s in range(_S_LOC):
                # both factors (s_in = 2s, 2s+1) in one DMA -> one wait sem
                t = in_pool.tile([_P, 2 * _FREE_IN], mybir.dt.float32)
                src = xr[:, 2 * s:2 * s + 2]  # (128, 2, 256), s-stride 32768
                dst = t[:, :].rearrange("p (s f) -> p s f", s=2)
                nc.sync.dma_start(out=dst, in_=src)
                in_tiles.append(t)

            ndma = 0
            for s in range(_S_LOC):
                # Pieces are (bl_start, bl_width, i_start, i_width) quarters of
                # the (bl, i) plane. Scope 0 ramps up from a tiny first piece so
                # the first output DMA issues as early as possible; later scopes
                # go out as single 4MB DMAs (large transfers sustain ~425 GB/s;
                # small ones pay ~1us of per-DMA boundary overhead).
                if s == 0:
                    pieces = [(0, 1, 0, 16), (0, 1, 16, 16), (1, 1, 0, _N),
                              (2, 2, 0, _N), (4, 4, 0, _N)]
                elif s in (1, 2, 3, 4):
                    pieces = [(0, 4, 0, _N), (4, 4, 0, _N)]
                else:
                    pieces = [(0, 8, 0, _N)]
                ot = out_pool.tile([_P, _FREE_OUT], mybir.dt.float32)
                dst = out[s].rearrange("d (bh bl) f -> (d bh) (bl f)", bh=_BH)
                for bl0, w, i0, wi in pieces:
                    if s == 0 and bl0 == 0:
                        src_t, off_a, off_b = ht, 0, _N
                    else:
                        src_t, off_a, off_b = in_tiles[s], bl0 * _N, _FREE_IN + bl0 * _N
                    # a: w bl-blocks of wi i-values (i-subrange only for w == 1)
                    a = src_t[:, off_a + i0:off_a + i0 + (w - 1) * _N + wi] \
                        .rearrange("p (bl i) -> p bl i", bl=w)
                    b = src_t[:, off_b:off_b + w * _N] \
                        .rearrange("p (bl j) -> p bl j", bl=w)
                    a4 = a.unsqueeze(3).broadcast_to([_P, w, wi, _N])
                    b4 = b.unsqueeze(2).broadcast_to([_P, w, wi, _N])
                    f0 = bl0 * _N * _N + i0 * _N
                    sz = w * wi * _N
                    osl = ot[:, f0:f0 + sz]
                    o4 = osl.rearrange("p (bl i j) -> p bl i j", bl=w, i=wi)
                    nc.vector.tensor_add(o4, a4, b4)
                    # Two HWDGE rings (SP=sync / ACT=scalar). The first three
                    # (tiny) pieces go on the scalar ring, which is empty while
                    # the input DMAs occupy the sync ring FIFO, so the output
                    # stream starts immediately. Every later DMA strictly
                    # alternates rings — with only one ring active, each DMA's
                    # ~1us completion boundary is exposed; alternation hides it
                    # under the other ring's data stream.
                    if ndma < 3:
                        eng = nc.scalar
                    else:
                        eng = nc.sync if ndma % 2 == 1 else nc.scalar
                    eng.dma_start(out=dst[:, f0:f0 + sz], in_=osl)
                    ndma += 1
    nc.compile()
    return nc


def kernel(x, num_factors):
    global LAST_RESULTS
    from concourse.bass_utils import run_bass_kernel_spmd

    x = np.asarray(x)
    assert x.shape == (_S_IN, _D, _B, _N), x.shape
    assert int(num_factors) == _NF, num_factors
    x = x.astype(np.float32, copy=False)

    if "nc" not in _CACHE:
        _CACHE["nc"] = _build_bass()
    nc = _CACHE["nc"]

    in_maps = [
        {"x": np.ascontiguousarray(x[c * _SIN_LOC:(c + 1) * _SIN_LOC])}
        for c in range(_N_CORES)
    ]
    res = run_bass_kernel_spmd(nc, in_maps, core_ids=list(range(_N_CORES)))
    LAST_RESULTS = res
    out = np.concatenate([res.results[c]["out"] for c in range(_N_CORES)], axis=0)
    return out.reshape(_S_OUT, _D, _B, _N ** _NF)



# revision 2
# speedup vs baseline: 1.7094x; 1.7094x over previous
"""Trainium2 Bass kernel for nn_DenseProduct (num_factors=2).

Computes, for input x of shape (128, 16, 64, 32) f32:
    out[s, d, b, i*32+j] = x[2s, d, b, i] + x[2s+1, d, b, j]
with output shape (64, 16, 64, 1024) f32.

Sharding: scope axis (dim 0) across 8 NeuronCores — core c handles output
scopes [8c, 8c+8).

The rel-err budget (2e-2) admits bf16, which halves the HBM write traffic
(the kernel is output-write bound: 256 MiB f32 -> 128 MiB bf16 total).

DVE 2x_1p perf mode requires every operand's innermost AP dim to be
stride +-1 with a 2-byte dtype. A plain broadcast outer-sum
    out[p,(bl,i,j)] = a[p,(bl,i)] + c[p,(bl,j)]
always leaves one operand with innermost stride 0. Instead iterate the
32x32 tile along wrap-around diagonals: with c doubled (c2 = [c|c]),
    out2[p, (bl, dd, t)] = a[p, (bl, t)] + c2[p, (bl, dd + t)]
every operand is innermost stride-1 (the stride-0 / stride-1-overlap dims
move to the middle), so the single tensor_tensor per scope runs at
2 elem/cycle/lane. out2 holds out_std[i=t, j=(dd+t)%32]; the host undoes
the diagonal permutation with one gather on the last axis.

Host side packs the input as rows of [a(32) | c(32) | c(32)] bf16 so a
scope's whole working set arrives as one contiguous DMA and the doubled-c
window needs no on-device prep.
"""

import numpy as np
import ml_dtypes

_S_IN = 128        # total input scopes
_NF = 2            # num_factors (hardcoded)
_S_OUT = _S_IN // _NF
_D = 16
_B = 64
_N = 32
_N_CORES = 8
_S_LOC = _S_OUT // _N_CORES    # 8 output scopes per core
_P = 128
_BH = 8
_BL = 8
_K = 3 * _N                    # 96 = a(32) | c(32) | c(32)
_FREE_IN = _BL * _K            # 768 per partition per scope
_FREE_OUT = _BL * _N * _N      # 8192 per partition per scope

_CACHE = {}
LAST_RESULTS = None  # BassKernelResults of the most recent run (for profiling)


def _diag_unperm():
    """index vector g: out_std[..., k] = out2[..., g[k]]."""
    k = np.arange(_N * _N)
    i = k // _N
    j = k % _N
    dd = (j - i) % _N
    return (dd * _N + i).astype(np.int64)


def _build_bass():
    import concourse.bacc as bacc
    import concourse.mybir as mybir
    from concourse.ap import AP
    from concourse.tile import TileContext

    nc = bacc.Bacc("TRN2", target_bir_lowering=False, debug=False,
                   num_devices=_N_CORES)
    x = nc.dram_tensor("x", [_S_LOC, _D, _B, _K], mybir.dt.bfloat16,
                       kind="ExternalInput").ap()
    out = nc.dram_tensor("out", [_S_LOC, _P, _FREE_OUT], mybir.dt.bfloat16,
                         kind="ExternalOutput").ap()

    with TileContext(nc) as tc:
        with tc.tile_pool(name="inp", bufs=_S_LOC) as in_pool, \
             tc.tile_pool(name="outp", bufs=4) as out_pool:
            # x[s, d, 8*bh+bl, k] -> partition (d, bh), free (s, (bl, k))
            xr = x.rearrange("s d (bh bl) k -> (d bh) s (bl k)", bh=_BH)
            in_tiles = []
            for s in range(_S_LOC):
                t = in_pool.tile([_P, _FREE_IN], mybir.dt.bfloat16)
                nc.sync.dma_start(out=t[:, :], in_=xr[:, s])
                in_tiles.append(t)

            ndma = 0
            for s in range(_S_LOC):
                # (bl0, w) pieces: scope 0 ramps up so the first output DMA
                # issues early; later scopes are single 2 MiB DMAs.
                if s == 0:
                    pieces = [(0, 1), (1, 1), (2, 2), (4, 4)]
                elif s in (1, 2):
                    pieces = [(0, 4), (4, 4)]
                else:
                    pieces = [(0, _BL)]
                ot = out_pool.tile([_P, _FREE_OUT], mybir.dt.bfloat16)
                src = in_tiles[s][:, :]
                pdim = list(src.ap[0])
                for bl0, w in pieces:
                    # out2[p, bl, dd, t] = a[p, bl, t] + c2[p, bl, dd + t]
                    a = AP(src.tensor, src.offset + bl0 * _K,
                           [pdim, [_K, w], [0, _N], [1, _N]])
                    c2 = AP(src.tensor, src.offset + bl0 * _K + _N,
                            [pdim, [_K, w], [1, _N], [1, _N]])
                    f0 = bl0 * _N * _N
                    sz = w * _N * _N
                    o4 = ot[:, f0:f0 + sz].rearrange(
                        "p (bl dd t) -> p bl dd t", bl=w, dd=_N)
                    nc.vector.tensor_add(o4, a, c2)
                    # Two HWDGE rings: first pieces on the scalar ring (the
                    # sync ring FIFO is busy with input DMAs), then strictly
                    # alternate so each DMA's ~1us completion boundary hides
                    # under the other ring's data stream.
                    if ndma < 3:
                        eng = nc.scalar
                    else:
                        eng = nc.sync if ndma % 2 == 1 else nc.scalar
                    eng.dma_start(out=out[s][:, f0:f0 + sz],
                                  in_=ot[:, f0:f0 + sz])
                    ndma += 1
    nc.compile()
    return nc


def kernel(x, num_factors):
    global LAST_RESULTS
    from concourse.bass_utils import run_bass_kernel_spmd

    x = np.asarray(x)
    assert x.shape == (_S_IN, _D, _B, _N), x.shape
    assert int(num_factors) == _NF, num_factors

    xb = x.astype(ml_dtypes.bfloat16)
    # [a | c | c] rows: scope s uses a = x[2s], c = x[2s+1]
    a = xb[0::2]
    c = xb[1::2]
    inp = np.concatenate([a, c, c], axis=-1)  # [64, 16, 64, 96]

    if "nc" not in _CACHE:
        _CACHE["nc"] = _build_bass()
        _CACHE["g"] = _diag_unperm()
    nc = _CACHE["nc"]

    in_maps = [
        {"x": np.ascontiguousarray(inp[cc * _S_LOC:(cc + 1) * _S_LOC])}
        for cc in range(_N_CORES)
    ]
    res = run_bass_kernel_spmd(nc, in_maps, core_ids=list(range(_N_CORES)))
    LAST_RESULTS = res
    raw = np.concatenate(
        [np.asarray(res.results[cc]["out"]) for cc in range(_N_CORES)], axis=0)
    # [64, P=(d, bh), (bl, dd, t)] -> [64, d, bh, bl, 1024(dd,t)]
    raw = raw.reshape(_S_OUT, _D, _BH, _BL, _N * _N)
    out = raw[..., _CACHE["g"]].astype(np.float32)
    return out.reshape(_S_OUT, _D, _B, _N ** _NF)


# revision 5
# speedup vs baseline: 1.7225x; 1.0076x over previous
"""Trainium2 Bass kernel for nn_DenseProduct (num_factors=2).

Computes, for input x of shape (128, 16, 64, 32) f32:
    out[s, d, b, i*32+j] = x[2s, d, b, i] + x[2s+1, d, b, j]
with output shape (64, 16, 64, 1024) f32.

Sharding: scope axis (dim 0) across 8 NeuronCores — core c handles output
scopes [8c, 8c+8).

The rel-err budget (2e-2) admits bf16, which halves the HBM write traffic
(the kernel is output-write bound: 256 MiB f32 -> 128 MiB bf16 total).

DVE 2x_1p perf mode requires every operand's innermost AP dim to be
stride +-1 with a 2-byte dtype. A plain broadcast outer-sum
    out[p,(bl,i,j)] = a[p,(bl,i)] + c[p,(bl,j)]
always leaves one operand with innermost stride 0. Instead iterate the
32x32 tile along wrap-around diagonals: with c doubled (c2 = [c|c]),
    out2[p, (bl, dd, t)] = a[p, (bl, t)] + c2[p, (bl, dd + t)]
every operand is innermost stride-1 (the stride-0 / stride-1-overlap dims
move to the middle), so the single tensor_tensor per scope runs at
2 elem/cycle/lane. out2 holds out_std[i=t, j=(dd+t)%32]; the host undoes
the diagonal permutation with one gather on the last axis.

Host side packs the input as rows of [a(32) | c(32) | c(32)] bf16 so a
scope's whole working set arrives as one contiguous DMA and the doubled-c
window needs no on-device prep.
"""

import numpy as np
import ml_dtypes

_S_IN = 128        # total input scopes
_NF = 2            # num_factors (hardcoded)
_S_OUT = _S_IN // _NF
_D = 16
_B = 64
_N = 32
_N_CORES = 8
_S_LOC = _S_OUT // _N_CORES    # 8 output scopes per core
_P = 128
_BH = 8
_BL = 8
_K = 3 * _N                    # 96 = a(32) | c(32) | c(32)
_FREE_IN = _BL * _K            # 768 per partition per scope
_FREE_OUT = _BL * _N * _N      # 8192 per partition per scope

_CACHE = {}
LAST_RESULTS = None  # BassKernelResults of the most recent run (for profiling)


def _diag_unperm():
    """index vector g: out_std[..., k] = out2[..., g[k]]."""
    k = np.arange(_N * _N)
    i = k // _N
    j = k % _N
    dd = (j - i) % _N
    return (dd * _N + i).astype(np.int64)


def _build_bass():
    import concourse.bacc as bacc
    import concourse.mybir as mybir
    from concourse.ap import AP
    from concourse.tile import TileContext

    nc = bacc.Bacc("TRN2", target_bir_lowering=False, debug=False,
                   num_devices=_N_CORES)
    x = nc.dram_tensor("x", [_S_LOC, _D, _B, _K], mybir.dt.bfloat16,
                       kind="ExternalInput").ap()
    out = nc.dram_tensor("out", [_S_LOC, _P, _FREE_OUT], mybir.dt.bfloat16,
                         kind="ExternalOutput").ap()

    with TileContext(nc) as tc:
        with tc.tile_pool(name="head", bufs=1) as head_pool, \
             tc.tile_pool(name="inp", bufs=2) as in_pool, \
             tc.tile_pool(name="inrest", bufs=1) as rest_pool, \
             tc.tile_pool(name="outp", bufs=4) as out_pool:
            # x[s, d, 8*bh+bl, k] -> partition (d, bh), free (s, (bl, k))
            xr = x.rearrange("s d (bh bl) k -> (d bh) s (bl k)", bh=_BH)
            # Input issue plan: each DMA_DIRECT2D occupies its issuing engine
            # ~0.6us, so don't serialize 8 issues on one engine. A tiny head
            # strip (scope 0, bl=0 only: 24 KB) lands first so the first TT
            # piece (and with it the output stream) starts as early as
            # possible; scopes 2-7 ride one batched DMA.
            head = head_pool.tile([_P, _K], mybir.dt.bfloat16)
            nc.sync.dma_start(out=head[:, :], in_=xr[:, 0][:, 0:_K])
            t0 = in_pool.tile([_P, _FREE_IN], mybir.dt.bfloat16)
            nc.scalar.dma_start(out=t0[:, :], in_=xr[:, 0])
            t1 = in_pool.tile([_P, _FREE_IN], mybir.dt.bfloat16)
            nc.sync.dma_start(out=t1[:, :], in_=xr[:, 1])
            trest = rest_pool.tile([_P, 6 * _FREE_IN], mybir.dt.bfloat16)
            nc.scalar.dma_start(
                out=trest[:, :].rearrange("p (s f) -> p s f", s=6),
                in_=xr[:, 2:_S_LOC])

            def in_src(s):
                if s == 0:
                    return t0[:, :]
                if s == 1:
                    return t1[:, :]
                return trest[:, (s - 2) * _FREE_IN:(s - 1) * _FREE_IN]

            ndma = 0
            for s in range(_S_LOC):
                # (bl0, w) pieces: scope 0 ramps up so the first output DMA
                # issues early; the last scope is split so the tail drain
                # after the final TT is 1 MiB, not 2.
                if s == 0:
                    pieces = [(0, 1), (1, 1), (2, 2), (4, 4)]
                elif s in (1, 2):
                    pieces = [(0, 4), (4, 4)]
                elif s == _S_LOC - 1:
                    pieces = [(0, 4), (4, 4)]
                else:
                    pieces = [(0, _BL)]
                ot = out_pool.tile([_P, _FREE_OUT], mybir.dt.bfloat16)
                for bl0, w in pieces:
                    if s == 0 and bl0 == 0:
                        src = head[:, :]
                        off = 0
                    else:
                        src = in_src(s)
                        off = bl0 * _K
                    pdim = list(src.ap[0])
                    # out2[p, bl, dd, t] = a[p, bl, t] + c2[p, bl, dd + t]
                    a = AP(src.tensor, src.offset + off,
                           [pdim, [_K, w], [0, _N], [1, _N]])
                    c2 = AP(src.tensor, src.offset + off + _N,
                            [pdim, [_K, w], [1, _N], [1, _N]])
                    f0 = bl0 * _N * _N
                    sz = w * _N * _N
                    o4 = ot[:, f0:f0 + sz].rearrange(
                        "p (bl dd t) -> p bl dd t", bl=w, dd=_N)
                    nc.vector.tensor_add(o4, a, c2)
                    # Two HWDGE rings. The scalar ring starts busy with the
                    # bulk input DMAs, so the first outputs go on sync; then
                    # strict alternation hides each DMA's ~1us completion
                    # boundary under the other ring's data stream.
                    if ndma < 3:
                        eng = nc.sync
                    else:
                        eng = nc.scalar if ndma % 2 == 1 else nc.sync
                    eng.dma_start(out=out[s][:, f0:f0 + sz],
                                  in_=ot[:, f0:f0 + sz])
                    ndma += 1
    nc.compile()
    return nc


def kernel(x, num_factors):
    global LAST_RESULTS
    from concourse.bass_utils import run_bass_kernel_spmd

    x = np.asarray(x)
    assert x.shape == (_S_IN, _D, _B, _N), x.shape
    assert int(num_factors) == _NF, num_factors

    xb = x.astype(ml_dtypes.bfloat16)
    # [a | c | c] rows: scope s uses a = x[2s], c = x[2s+1]
    a = xb[0::2]
    c = xb[1::2]
    inp = np.concatenate([a, c, c], axis=-1)  # [64, 16, 64, 96]

    if "nc" not in _CACHE:
        _CACHE["nc"] = _build_bass()
        _CACHE["g"] = _diag_unperm()
    nc = _CACHE["nc"]

    in_maps = [
        {"x": np.ascontiguousarray(inp[cc * _S_LOC:(cc + 1) * _S_LOC])}
        for cc in range(_N_CORES)
    ]
    res = run_bass_kernel_spmd(nc, in_maps, core_ids=list(range(_N_CORES)))
    LAST_RESULTS = res
    raw = np.concatenate(
        [np.asarray(res.results[cc]["out"]) for cc in range(_N_CORES)], axis=0)
    # [64, P=(d, bh), (bl, dd, t)] -> [64, d, bh, bl, 1024(dd,t)]
    raw = raw.reshape(_S_OUT, _D, _BH, _BL, _N * _N)
    out = raw[..., _CACHE["g"]].astype(np.float32)
    return out.reshape(_S_OUT, _D, _B, _N ** _NF)
